# revision 63
# baseline (speedup 1.0000x reference)
"""Multi-head self-attention on 8 trn2 NeuronCores.

Problem: B=4, S=2048, E=1024, H=8, D=128 MHA with a boolean attention mask.

Sharding: batch x head-group. Core c computes batch b=c//2 for heads
[4*(c%2), 4*(c%2)+4). Each core produces a partial output [S, E] (its 4
heads' contribution through w_out); the host sums the two partials per
batch. No on-device collectives needed.

Single interleaved pipeline (no phase barriers): projections of head h+1
and the output projection are software-pipelined into the attention of
head h as PE "filler" work, so the PE stays dense (which also keeps its
DVFS p-state at full clock). The softmax denominator is NOT computed
with ones-matmuls per key tile (a third of the attention PE time);
instead the masked exp tiles are accumulated across the 16 key tiles on
the DVE (bf16) and reduced with two tiny ones-matmuls per query-pair.

All DRAM tensors are pre-swizzled on the host into partition-major
layouts so every DMA is 128 contiguous descriptors of >=2KB (the
naive layouts were descriptor-bound at 256B/descriptor).

Per (head, query-pair of 1024), streaming over 16 key tiles of 128:
  lgT[128k, 1024q] = KT-tile.T @ QT        (PE)
  expT = exp(scale * lgT)                  (ACT, bf16 out)
  expT *= keepT-tile                       (DVE; masked keys -> 0)
  acc += expT                              (DVE only, bf16, wide 2-kt ops)
  av  += V-tile.T @ expT                   (PE, [128d, 512q] x2, one kt behind)
  tail: avs = av (ACT/DVE), sm = ones.T @ acc (PE, [1,512]x2 in one bank
    via partition offset 32); deferred one pair: rcb = exp(-ln(sm)) (ACT),
    pb = ones.T @ rcb broadcast (PE), headsT = avs * pb (DVE).
Output projection: out[128q, E] = sum_h headsT[h].T @ w_out[h], staged
through SBUF, DMA'd per [128, 512] chunk.

exp is computed without a running row-max: logits here are ~N(0, 2.7^2), so
exp stays well inside fp32 range and softmax is shift invariant.
"""

import math

import ml_dtypes
import numpy as np

import concourse.bass as bass
import concourse.tile as tile
from concourse import mybir
from concourse.bass_utils import run_bass_kernel_spmd
from concourse.masks import make_identity
from concourse.vector_clock import ScopedClock, VectorClock

B, S, E, H, D = 4, 2048, 1024, 8, 128
HPC = 4          # heads per core
NCORES = 8
NKT = S // 128   # key tiles per sequence
NET = E // 128   # contraction tiles for the projections
SCALE = 1.0 / math.sqrt(D)
BF16 = mybir.dt.bfloat16
F32 = mybir.dt.float32
EXP = mybir.ActivationFunctionType.Exp
LN = mybir.ActivationFunctionType.Ln

FILL_PER_KT = 3          # PE filler thunks pulled per key tile
# exp lands in double-wide [128, 2, 1024] tiles (one per 2 key tiles), so
# mask and acc run as half as many, twice as wide DVE ops. POOL_JS selects
# ex2 tiles whose acc-add would run on GpSimd — left EMPTY on purpose: a
# GpSimd wide tensor op running concurrently with a DVE mask slows BOTH ~3x
# via SBUF port contention (measured 1.2us -> 4.1us), which cascades into a
# PE stall; a single all-DVE chain is ~35us faster end-to-end.
POOL_JS = frozenset()

_patched = False


def _patch_drain():
    """The installed walrus rejects >1 sem wait on the Tile tail Drain.
    Emit one drain per pending logical processor instead."""
    global _patched
    if _patched:
        return
    _patched = True

    def _drain_and_barrier(self, tick_clock, wait_clock):
        nc = self.nc
        ticks = list(tick_clock.global_clock)
        procs = [i for i, t in enumerate(ticks) if t > 0]
        for p in procs or [None]:
            vec = [0] * len(ticks)
            if p is not None:
                vec[p] = ticks[p]
            d = nc.sync.drain()
            wait_clock.add_sem_waits(d.ins, ScopedClock({None: VectorClock(vec)}))
        nc.all_engine_barrier()
        popped = nc._tile_sem_poison_stack.pop()
        assert popped is self._sem_poison
        nc.clear_and_free_semaphores(list(self.sems.allocated().values()))
        nc.all_engine_barrier()

    tile.TileContext._drain_and_barrier = _drain_and_barrier


def _split_waits(nc):
    """This walrus build only encodes ONE sem wait per instruction. Move
    extra waits onto preceding same-engine NoOps (engines execute their
    instructions in block order, so this is semantically identical)."""
    import bass_rust

    k = 0
    for f in nc.m.functions:
        for bb in f.blocks:
            out = []
            for inst in bb.instructions:
                si = inst.sync_info
                if si is not None and si.on_wait and len(si.on_wait) > 1:
                    waits = list(si.on_wait)
                    for w in waits[:-1]:
                        nop = bass_rust.InstNoOp(
                            name=f"I-waitsplit-{k}", ins=[], outs=[]
                        )
                        k += 1
                        nop.engine = inst.engine
                        nop.sync_info = mybir.SyncInfo(on_wait=[w], on_update=[])
                        out.append(nop)
                    inst.sync_info = mybir.SyncInfo(
                        on_wait=[waits[-1]], on_update=si.on_update
                    )
                out.append(inst)
            bb.instructions[:] = out


_nc_cache = None


def _build_nc():
    global _nc_cache
    if _nc_cache is not None:
        return _nc_cache
    _patch_drain()

    nc = bass.Bass()
    # host-pre-swizzled, partition-major layouts (see kernel() below)
    qT_d = nc.declare_dram_parameter("qT", [128, 4, NET, 512], BF16, isOutput=False)
    keepT_d = nc.declare_dram_parameter("keepT", [128, NKT, S], BF16, isOutput=False)
    wq_d = nc.declare_dram_parameter("wq", [128, HPC, NET, D], BF16, isOutput=False)
    wk_d = nc.declare_dram_parameter("wk", [128, HPC, NET, D], BF16, isOutput=False)
    wv_d = nc.declare_dram_parameter("wv", [128, HPC, NET, D], BF16, isOutput=False)
    wo_d = nc.declare_dram_parameter("wo", [128, HPC, E], BF16, isOutput=False)
    out_d = nc.declare_dram_parameter("out", [S, E], F32, isOutput=True)
    w_d = {"wq": wq_d, "wk": wk_d, "wv": wv_d}

    with tile.TileContext(nc) as tc:
        with (
            tc.tile_pool(name="const", bufs=1) as constp,
            tc.tile_pool(name="wos", bufs=1) as wop,
            tc.tile_pool(name="hT", bufs=1) as hTp,
            tc.tile_pool(name="qTs", bufs=1) as qTp,
            tc.tile_pool(name="keep", bufs=1) as keepp,
            tc.tile_pool(name="ws", bufs=1) as wsp,
            tc.tile_pool(name="qkv", bufs=2) as qkvp,
            tc.tile_pool(name="vt", bufs=1) as vtp,
            tc.tile_pool(name="expt", bufs=3) as expp,
            tc.tile_pool(name="accs", bufs=2) as accp,
            tc.tile_pool(name="avs", bufs=4) as avsp,
            tc.tile_pool(name="small", bufs=2) as smallp,
            tc.tile_pool(name="obs", bufs=2) as obp,
            tc.tile_pool(name="ps_lg", bufs=2, space="PSUM") as ps_lg,
            tc.tile_pool(name="ps_av", bufs=1, space="PSUM") as ps_av,
            tc.tile_pool(name="ps_pr", bufs=1, space="PSUM") as ps_pr,
            tc.tile_pool(name="ps_sm", bufs=1, space="PSUM") as ps_sm,
        ):
            # ---- constants ----
            ident = constp.tile([128, 128], BF16)
            make_identity(nc, ident)
            ones_col = constp.tile([128, 1], BF16)
            nc.vector.memset(ones_col, 1.0)
            ones33 = constp.tile([33, 128], BF16)
            nc.vector.memset(ones33, 1.0)

            wo_s = wop.tile([128, HPC, E], BF16)
            headsT_s = hTp.tile([128, HPC, S], BF16)
            qT_s = qTp.tile([128, 4, NET, 512], BF16)
            keep_s = keepp.tile([128, NKT, S], BF16)
            w_s = {
                name: wsp.tile([128, HPC, NET, D], BF16, tag=name, name=name)
                for name in ("wq", "wk", "wv")
            }

            def qT_sl(c):
                # global s-columns [512c, 512c+512) in the quarter-major layout
                return qT_s[:, c]

            # ---- prefetch DMAs (sync queue: weights+qT+wo; pool queue: keepT)
            def load_w(name, h):
                nc.sync.dma_start(out=w_s[name][:, h], in_=w_d[name][:, h])

            load_w("wk", 0)
            nc.sync.dma_start(out=qT_s[:, 0], in_=qT_d[:, 0])
            load_w("wv", 0)
            load_w("wq", 0)
            for qu in range(1, 4):
                nc.sync.dma_start(out=qT_s[:, qu], in_=qT_d[:, qu])
            for h in range(1, HPC):
                for name in ("wk", "wv", "wq"):
                    load_w(name, h)
                if h == 1:
                    nc.sync.dma_start(out=wo_s, in_=wo_d[:, :, :])
            # hold the whole mask stream until the projection-critical qT
            # quarters are in (shared HBM bandwidth; keepT isn't needed until
            # attention starts ~45us in). The probes just make the Pool DMA
            # queue wait on the respective qT quarter's arrival.
            probe = constp.tile([1, 8], BF16, name="probe")
            nc.gpsimd.tensor_copy(probe, qT_s[0:1, 1, 0, 0:8])
            for kt in range(4):
                nc.gpsimd.dma_start(out=keep_s[:, kt, :], in_=keepT_d[:, kt, :])
            probe2 = constp.tile([1, 8], BF16, name="probe2")
            nc.gpsimd.tensor_copy(probe2, qT_s[0:1, 3, 0, 0:8])
            for kt in range(4, NKT):
                nc.gpsimd.dma_start(out=keep_s[:, kt, :], in_=keepT_d[:, kt, :])

            # ---- filler queue: PE work to interleave into attention ----
            filler = []
            fill_pos = [0]

            def pull(n):
                ran = 0
                while ran < n and fill_pos[0] < len(filler):
                    filler[fill_pos[0]][1]()
                    fill_pos[0] += 1
                    ran += 1

            def drain(stage):
                while fill_pos[0] < len(filler) and filler[fill_pos[0]][0] <= stage:
                    filler[fill_pos[0]][1]()
                    fill_pos[0] += 1

            evac_flip = [0]

            def evac(dst, src):
                if evac_flip[0] == 0:
                    nc.scalar.copy(dst, src)
                else:
                    nc.vector.tensor_copy(dst, src)
                evac_flip[0] ^= 1

            qkv = {}

            def enqueue_proj(h):
                QT_t = qkvp.tile([128, S], BF16, tag="QT", name=f"QT{h}")
                KT_t = qkvp.tile([128, S], BF16, tag="KT", name=f"KT{h}")
                V_t = qkvp.tile([128, NKT, 128], BF16, tag="V", name=f"V{h}")
                vt_t = vtp.tile([128, S], BF16, tag="vt", name=f"vt{h}")
                qkv[h] = (QT_t, KT_t, V_t)

                def chunk(wname, c, dst):
                    hold = {}
                    ths = []
                    for et in range(NET):
                        def th(et=et, wname=wname, c=c, dst=dst, hold=hold):
                            if et == 0:
                                hold["ps"] = ps_pr.tile(
                                    [128, 512], F32, tag="pr", name=f"pp{h}"
                                )
                            nc.tensor.matmul(
                                hold["ps"],
                                lhsT=w_s[wname][:, h, et, :],
                                rhs=qT_sl(c)[:, et, :],
                                start=(et == 0),
                                stop=(et == NET - 1),
                            )
                            if et == NET - 1:
                                evac(dst[:, c * 512 : (c + 1) * 512], hold["ps"])
                        ths.append(th)
                    return ths

                def pst_group(g):
                    hold = {}
                    ths = []
                    for j in range(8):
                        def th(j=j, g=g, hold=hold):
                            if j == 0:
                                hold["ps"] = ps_pr.tile(
                                    [128, 8, 128], BF16, tag="pr", name=f"pt{h}"
                                )
                            st = 8 * g + j
                            nc.tensor.transpose(
                                hold["ps"][:, j, :],
                                vt_t[:, st * 128 : (st + 1) * 128],
                                ident,
                            )
                            if j == 7:
                                nc.vector.tensor_copy(
                                    V_t[:, 8 * g : 8 * g + 8, :], hold["ps"]
                                )
                        ths.append(th)
                    return ths

                seq = []
                for c in range(4):
                    seq += chunk("wk", c, KT_t)
                    seq += chunk("wv", c, vt_t)
                    seq += chunk("wq", c, QT_t)
                    if c == 1:
                        seq += pst_group(0)
                seq += pst_group(1)
                for th in seq:
                    filler.append((h, th))

            def enqueue_out(qts, stage, wide):
                # wide=False: [128,512] chunks through the single-buffer proj
                # bank (safe to pull as filler inside attention pairs).
                # wide=True: [128,1024] through the double-buffered lg ring.
                # Evacuation is LAZY (emitted just before the next qt's
                # alloc) so the ring never blocks the next qt's matmuls.
                lazy = {}

                def flush():
                    if "po" in lazy:
                        po, qt_, half_, nmm_ = lazy.pop("po")
                        for m in range(nmm_):
                            e0 = half_ * (1024 if wide else 512) + m * 512
                            ob = obp.tile([128, 512], F32, tag="ob", name="ob")
                            evac(ob, po[:, m * 512 : (m + 1) * 512])
                            nc.sync.dma_start(
                                out=out_d[
                                    qt_ * 128 : (qt_ + 1) * 128, e0 : e0 + 512
                                ],
                                in_=ob,
                            )

                for qt in qts:
                    for half in range(1 if wide else 2):
                        hold = {}
                        for hh in range(HPC):
                            def th(hh=hh, qt=qt, half=half, hold=hold):
                                nmm = 2 if wide else 1
                                if hh == 0:
                                    flush()
                                    hold["po"] = (
                                        ps_lg.tile([128, 1024], F32, tag="lg", name="po")
                                        if wide
                                        else ps_pr.tile([128, 512], F32, tag="pr", name="po")
                                    )
                                for m in range(nmm):
                                    e0 = half * (1024 if wide else 512) + m * 512
                                    nc.tensor.matmul(
                                        hold["po"][:, m * 512 : (m + 1) * 512],
                                        lhsT=headsT_s[:, hh, qt * 128 : (qt + 1) * 128],
                                        rhs=wo_s[:, hh, e0 : e0 + 512],
                                        start=(hh == 0),
                                        stop=(hh == HPC - 1),
                                    )
                                if hh == HPC - 1:
                                    lazy["po"] = (hold["po"], qt, half, nmm)
                            filler.append((stage, th))
                filler.append((stage, flush))

            # ---- deferred softmax normalization (one pair behind) ----
            def emit_norm(pn):
                sm, avs0, avs1, hh, q0 = pn
                lns = smallp.tile([33, 512], F32, tag="lns", name="lns", bufs=1)
                nc.scalar.activation(lns, sm, LN)
                rcb = smallp.tile([33, 512], BF16, tag="rcb", name="rcb")
                nc.scalar.activation(rcb, lns, EXP, scale=-1.0)
                pb = ps_lg.tile([128, 2, 512], F32, tag="lg", name="pb")
                nc.tensor.matmul(
                    pb[:, 0, :], lhsT=ones33[0:1, :], rhs=rcb[0:1, :],
                    start=True, stop=True,
                )
                nc.tensor.matmul(
                    pb[:, 1, :], lhsT=ones33[32:33, :], rhs=rcb[32:33, :],
                    start=True, stop=True,
                )
                rb = smallp.tile([128, 2, 512], BF16, tag="rb", name="rb", bufs=1)
                nc.scalar.copy(rb, pb)
                nc.vector.tensor_mul(headsT_s[:, hh, q0 : q0 + 512], avs0, rb[:, 0, :])
                nc.vector.tensor_mul(
                    headsT_s[:, hh, q0 + 512 : q0 + 1024], avs1, rb[:, 1, :]
                )

            # the softmax denominator reduction (ones-matmuls -> sm) and the
            # normalization are BOTH deferred: ones(p) runs at the start of
            # pair p+1 (its accumulators are a full pair old, so the PE never
            # waits on the acc chains), norm(p) at the start of pair p+2.
            ones_pending = [None]  # (acc_d, acc_p, avs0, avs1, h, q0)
            norm_pending = [None]  # (sm, avs0, avs1, h, q0)

            def emit_ones(op):
                acc_d, acc_p, avs0, avs1, hh, q0 = op
                sm = ps_sm.tile([33, 512], F32, tag="sm", name="sm")
                for row0, sl in ((0, slice(0, 512)), (32, slice(512, 1024))):
                    srcs = [a[:, i, sl] for a in (acc_d, acc_p) if a is not None for i in range(2)]
                    for si, src in enumerate(srcs):
                        nc.tensor.matmul(
                            sm[row0 : row0 + 1, :], lhsT=ones_col, rhs=src,
                            start=(si == 0), stop=(si == len(srcs) - 1),
                        )
                return (sm, avs0, avs1, hh, q0)

            def attention_pair(h, pair):
                QT_t, KT_t, V_t = qkv[h]
                q0 = pair * 1024
                norm_next = None
                if ones_pending[0] is not None:
                    norm_next = emit_ones(ones_pending[0])
                    ones_pending[0] = None
                norm_now = norm_pending[0]  # emitted at kt==2, see below
                norm_pending[0] = norm_next
                if h == HPC - 1 and pair == 1:
                    # first half of the output projection needs the last
                    # head's pair-0 norm before its thunks become pullable
                    if norm_now is not None:
                        emit_norm(norm_now)
                        norm_now = None
                    enqueue_out(range(8), stage=HPC, wide=False)
                ex2_of = {}   # kt -> (ex2 tile, sub-index)
                ex2s = {}     # j -> ex2 tile
                acc_d = None
                acc_p = None
                av = [None]
                av_started = [False]
                av_queue = []  # (kt, due_iter)

                def emit_av(kt, stop):
                    if av[0] is None:
                        av[0] = ps_av.tile([128, 2, 512], F32, tag="av", name="av")
                    t, i = ex2_of[kt]
                    for sub in range(2):
                        nc.tensor.matmul(
                            av[0][:, sub, :],
                            lhsT=V_t[:, kt, :],
                            rhs=t[:, i, sub * 512 : (sub + 1) * 512],
                            start=not av_started[0],
                            stop=stop,
                        )
                    av_started[0] = True

                for kt in range(NKT):
                    if kt == 2 and norm_now is not None:
                        # deferred here so the norm's ACT ops queue behind
                        # exp(0..1) instead of blocking them at pair start
                        emit_norm(norm_now)
                        norm_now = None
                    j = kt // 2
                    lg = ps_lg.tile([128, 1024], F32, tag="lg", name="lg")
                    for half in range(2):
                        nc.tensor.matmul(
                            lg[:, half * 512 : (half + 1) * 512],
                            lhsT=KT_t[:, kt * 128 : (kt + 1) * 128],
                            rhs=QT_t[:, q0 + half * 512 : q0 + (half + 1) * 512],
                            start=True,
                            stop=True,
                        )
                    if kt % 2 == 0:
                        ex2s[j] = expp.tile([128, 2, 1024], BF16, tag="ex", name="ex")
                    ex2 = ex2s[j]
                    nc.scalar.activation(ex2[:, kt % 2, :], lg, EXP, scale=SCALE)
                    ex2_of[kt] = (ex2, kt % 2)
                    if kt % 2 == 1:
                        # one wide mask-mul covers both key tiles of this ex2
                        nc.vector.tensor_mul(
                            ex2, ex2, keep_s[:, kt - 1 : kt + 1, q0 : q0 + 1024]
                        )
                        if j in POOL_JS:
                            if acc_p is None:
                                acc_p = accp.tile(
                                    [128, 2, 1024], BF16, tag="accp", name="accp", bufs=1
                                )
                                nc.gpsimd.tensor_add(acc_p, ex2s[j - 1], ex2)
                            else:
                                nc.gpsimd.tensor_add(acc_p, acc_p, ex2)
                        elif (POOL_JS and j == min(POOL_JS) - 1) or j == 0:
                            pass  # consumed by its chain's init later
                        elif j == 1:
                            acc_d = accp.tile(
                                [128, 2, 1024], BF16, tag="accd", name="accd", bufs=1
                            )
                            nc.vector.tensor_add(acc_d, ex2s[0], ex2)
                        else:
                            nc.vector.tensor_add(acc_d, acc_d, ex2)
                    pull(2 if kt < 10 else 4)
                    while av_queue and av_queue[0][1] <= kt:
                        emit_av(av_queue.pop(0)[0], stop=False)
                    av_queue.append((kt, kt + 1 if kt % 2 else kt + 2))
                while len(av_queue) > 1:
                    emit_av(av_queue.pop(0)[0], stop=False)
                emit_av(av_queue.pop(0)[0], stop=True)
                pull(20)
                avs0 = avsp.tile([128, 512], BF16, tag="avs", name="avs0")
                nc.scalar.copy(avs0, av[0][:, 0, :])
                avs1 = avsp.tile([128, 512], BF16, tag="avs", name="avs1")
                nc.vector.tensor_copy(avs1, av[0][:, 1, :])
                pack = (acc_d, acc_p, avs0, avs1, h, q0)
                if 2 * h + pair == 6:
                    # pair 6 is (h3,p0): its sm must exist before pair 7's
                    # start (which enqueues the first output-proj batch), so
                    # run its ones at its own tail. Consume the pending norm
                    # first so the single sm PSUM slot is free.
                    if norm_pending[0] is not None:
                        emit_norm(norm_pending[0])
                    norm_pending[0] = emit_ones(pack)
                else:
                    ones_pending[0] = pack
                pull(2)

            # ---- the pipeline ----
            enqueue_proj(0)
            drain(0)
            for h in range(HPC):
                if h + 1 < HPC:
                    enqueue_proj(h + 1)
                for pair in (0, 1):
                    attention_pair(h, pair)
                if h + 1 < HPC:
                    drain(h + 1)
            emit_norm(emit_ones(ones_pending[0]))
            enqueue_out(range(8, 16), stage=HPC + 1, wide=True)
            drain(HPC + 1)

    _split_waits(nc)
    _nc_cache = nc
    return nc


def kernel(q, mask, w_query, w_key, w_value, w_out):
    nc = _build_nc()
    bf16 = ml_dtypes.bfloat16

    # partition-major swizzles so every DMA is 128 contiguous big descriptors
    qTp = np.empty((B, 128, 4, NET, 512), dtype=bf16)
    keepTp = np.empty((B, 128, NKT, S), dtype=bf16)
    for b in range(B):
        qT = np.ascontiguousarray(q[b].T.astype(bf16))  # [E, S]
        qTp[b] = qT.reshape(NET, 128, 4, 512).transpose(1, 2, 0, 3)
        keepT = (~mask[b]).T.astype(bf16)  # [S(k), S(q)]
        keepTp[b] = keepT.reshape(NKT, 128, S).transpose(1, 0, 2)
    wp = {}
    for name, w in (("wq", w_query), ("wk", w_key), ("wv", w_value)):
        wp[name] = np.ascontiguousarray(
            w.astype(bf16).reshape(H, NET, 128, D).transpose(2, 0, 1, 3)
        )
    wop = np.ascontiguousarray(w_out.astype(bf16).transpose(1, 0, 2))  # [128, H, E]

    in_maps = []
    for c in range(NCORES):
        b, g = c // 2, c % 2
        hs = slice(g * HPC, (g + 1) * HPC)
        in_maps.append(
            {
                "qT": np.ascontiguousarray(qTp[b]),
                "keepT": np.ascontiguousarray(keepTp[b]),
                "wq": np.ascontiguousarray(wp["wq"][:, hs]),
                "wk": np.ascontiguousarray(wp["wk"][:, hs]),
                "wv": np.ascontiguousarray(wp["wv"][:, hs]),
                "wo": np.ascontiguousarray(wop[:, hs]),
            }
        )

    global _last_in_maps
    _last_in_maps = in_maps
    res = run_bass_kernel_spmd(nc, in_maps, list(range(NCORES)))
    outs = [r["out"] for r in res.results]
    return np.stack([outs[2 * b] + outs[2 * b + 1] for b in range(B)]).astype(
        np.float32
    )


# revision 64
# speedup vs baseline: 1.0045x; 1.0045x over previous
"""Multi-head self-attention on 8 trn2 NeuronCores.

Problem: B=4, S=2048, E=1024, H=8, D=128 MHA with a boolean attention mask.

Sharding: batch x head-group. Core c computes batch b=c//2 for heads
[4*(c%2), 4*(c%2)+4). Each core produces a partial output [S, E] (its 4
heads' contribution through w_out); the host sums the two partials per
batch. No on-device collectives needed.

Single interleaved pipeline (no phase barriers): projections of head h+1
and the output projection are software-pipelined into the attention of
head h as PE "filler" work, so the PE stays dense (which also keeps its
DVFS p-state at full clock). The softmax denominator is NOT computed
with ones-matmuls per key tile (a third of the attention PE time);
instead the masked exp tiles are accumulated across the 16 key tiles on
the DVE (bf16) and reduced with two tiny ones-matmuls per query-pair.

All DRAM tensors are pre-swizzled on the host into partition-major
layouts so every DMA is 128 contiguous descriptors of >=2KB (the
naive layouts were descriptor-bound at 256B/descriptor).

Per (head, query-pair of 1024), streaming over 16 key tiles of 128:
  lgT[128k, 1024q] = KT-tile.T @ QT        (PE)
  expT = exp(scale * lgT)                  (ACT, bf16 out)
  expT *= keepT-tile                       (DVE; masked keys -> 0)
  acc += expT                              (DVE only, bf16, wide 2-kt ops)
  av  += V-tile.T @ expT                   (PE, [128d, 512q] x2, one kt behind)
  tail: avs = av (ACT/DVE), sm = ones.T @ acc (PE, [1,512]x2 in one bank
    via partition offset 32); deferred one pair: rcb = exp(-ln(sm)) (ACT),
    pb = ones.T @ rcb broadcast (PE), headsT = avs * pb (DVE).
Output projection: out[128q, E] = sum_h headsT[h].T @ w_out[h], staged
through SBUF, DMA'd per [128, 512] chunk.

exp is computed without a running row-max: logits here are ~N(0, 2.7^2), so
exp stays well inside fp32 range and softmax is shift invariant.
"""

import math

import ml_dtypes
import numpy as np

import concourse.bass as bass
import concourse.tile as tile
from concourse import mybir
from concourse.bass_utils import run_bass_kernel_spmd
from concourse.masks import make_identity
from concourse.vector_clock import ScopedClock, VectorClock

B, S, E, H, D = 4, 2048, 1024, 8, 128
HPC = 4          # heads per core
NCORES = 8
NKT = S // 128   # key tiles per sequence
NET = E // 128   # contraction tiles for the projections
SCALE = 1.0 / math.sqrt(D)
BF16 = mybir.dt.bfloat16
F32 = mybir.dt.float32
EXP = mybir.ActivationFunctionType.Exp
LN = mybir.ActivationFunctionType.Ln

FILL_PER_KT = 3          # PE filler thunks pulled per key tile
# exp lands in double-wide [128, 2, 1024] tiles (one per 2 key tiles), so
# mask and acc run as half as many, twice as wide DVE ops. POOL_JS selects
# ex2 tiles whose acc-add would run on GpSimd — left EMPTY on purpose: a
# GpSimd wide tensor op running concurrently with a DVE mask slows BOTH ~3x
# via SBUF port contention (measured 1.2us -> 4.1us), which cascades into a
# PE stall; a single all-DVE chain is ~35us faster end-to-end.
POOL_JS = frozenset()

_patched = False


def _patch_drain():
    """The installed walrus rejects >1 sem wait on the Tile tail Drain.
    Emit one drain per pending logical processor instead."""
    global _patched
    if _patched:
        return
    _patched = True

    def _drain_and_barrier(self, tick_clock, wait_clock):
        nc = self.nc
        ticks = list(tick_clock.global_clock)
        procs = [i for i, t in enumerate(ticks) if t > 0]
        for p in procs or [None]:
            vec = [0] * len(ticks)
            if p is not None:
                vec[p] = ticks[p]
            d = nc.sync.drain()
            wait_clock.add_sem_waits(d.ins, ScopedClock({None: VectorClock(vec)}))
        nc.all_engine_barrier()
        popped = nc._tile_sem_poison_stack.pop()
        assert popped is self._sem_poison
        nc.clear_and_free_semaphores(list(self.sems.allocated().values()))
        nc.all_engine_barrier()

    tile.TileContext._drain_and_barrier = _drain_and_barrier


def _split_waits(nc):
    """This walrus build only encodes ONE sem wait per instruction. Move
    extra waits onto preceding same-engine NoOps (engines execute their
    instructions in block order, so this is semantically identical)."""
    import bass_rust

    k = 0
    for f in nc.m.functions:
        for bb in f.blocks:
            out = []
            for inst in bb.instructions:
                si = inst.sync_info
                if si is not None and si.on_wait and len(si.on_wait) > 1:
                    waits = list(si.on_wait)
                    for w in waits[:-1]:
                        nop = bass_rust.InstNoOp(
                            name=f"I-waitsplit-{k}", ins=[], outs=[]
                        )
                        k += 1
                        nop.engine = inst.engine
                        nop.sync_info = mybir.SyncInfo(on_wait=[w], on_update=[])
                        out.append(nop)
                    inst.sync_info = mybir.SyncInfo(
                        on_wait=[waits[-1]], on_update=si.on_update
                    )
                out.append(inst)
            bb.instructions[:] = out


_nc_cache = None


def _build_nc():
    global _nc_cache
    if _nc_cache is not None:
        return _nc_cache
    _patch_drain()

    nc = bass.Bass()
    # host-pre-swizzled, partition-major layouts (see kernel() below)
    qT_d = nc.declare_dram_parameter("qT", [128, 4, NET, 512], BF16, isOutput=False)
    keepT_d = nc.declare_dram_parameter("keepT", [128, NKT, S], BF16, isOutput=False)
    wq_d = nc.declare_dram_parameter("wq", [128, HPC, NET, D], BF16, isOutput=False)
    wk_d = nc.declare_dram_parameter("wk", [128, HPC, NET, D], BF16, isOutput=False)
    wv_d = nc.declare_dram_parameter("wv", [128, HPC, NET, D], BF16, isOutput=False)
    wo_d = nc.declare_dram_parameter("wo", [128, HPC, E], BF16, isOutput=False)
    out_d = nc.declare_dram_parameter("out", [S, E], F32, isOutput=True)
    w_d = {"wq": wq_d, "wk": wk_d, "wv": wv_d}

    with tile.TileContext(nc) as tc:
        with (
            tc.tile_pool(name="const", bufs=1) as constp,
            tc.tile_pool(name="wos", bufs=1) as wop,
            tc.tile_pool(name="hT", bufs=1) as hTp,
            tc.tile_pool(name="qTs", bufs=1) as qTp,
            tc.tile_pool(name="keep", bufs=1) as keepp,
            tc.tile_pool(name="ws", bufs=1) as wsp,
            tc.tile_pool(name="qkv", bufs=2) as qkvp,
            tc.tile_pool(name="vt", bufs=1) as vtp,
            tc.tile_pool(name="expt", bufs=3) as expp,
            tc.tile_pool(name="accs", bufs=2) as accp,
            tc.tile_pool(name="avs", bufs=4) as avsp,
            tc.tile_pool(name="small", bufs=2) as smallp,
            tc.tile_pool(name="obs", bufs=2) as obp,
            tc.tile_pool(name="ps_lg", bufs=2, space="PSUM") as ps_lg,
            tc.tile_pool(name="ps_av", bufs=1, space="PSUM") as ps_av,
            tc.tile_pool(name="ps_pr", bufs=1, space="PSUM") as ps_pr,
            tc.tile_pool(name="ps_sm", bufs=1, space="PSUM") as ps_sm,
        ):
            # ---- constants ----
            ident = constp.tile([128, 128], BF16)
            make_identity(nc, ident)
            ones_col = constp.tile([128, 1], BF16)
            nc.vector.memset(ones_col, 1.0)
            ones33 = constp.tile([33, 128], BF16)
            nc.vector.memset(ones33, 1.0)

            wo_s = wop.tile([128, HPC, E], BF16)
            headsT_s = hTp.tile([128, HPC, S], BF16)
            qT_s = qTp.tile([128, 4, NET, 512], BF16)
            keep_s = keepp.tile([128, NKT, S], BF16)
            w_s = {
                name: wsp.tile([128, HPC, NET, D], BF16, tag=name, name=name)
                for name in ("wq", "wk", "wv")
            }

            def qT_sl(c):
                # global s-columns [512c, 512c+512) in the quarter-major layout
                return qT_s[:, c]

            # ---- prefetch DMAs (sync queue: weights+qT+wo; pool queue: keepT)
            def load_w(name, h):
                nc.sync.dma_start(out=w_s[name][:, h], in_=w_d[name][:, h])

            load_w("wk", 0)
            nc.sync.dma_start(out=qT_s[:, 0], in_=qT_d[:, 0])
            load_w("wv", 0)
            load_w("wq", 0)
            for qu in range(1, 4):
                nc.sync.dma_start(out=qT_s[:, qu], in_=qT_d[:, qu])
            for h in range(1, HPC):
                for name in ("wk", "wv", "wq"):
                    load_w(name, h)
                if h == 1:
                    nc.sync.dma_start(out=wo_s, in_=wo_d[:, :, :])
            # hold the whole mask stream until the projection-critical qT
            # quarters are in (shared HBM bandwidth; keepT isn't needed until
            # attention starts ~45us in). The probes just make the Pool DMA
            # queue wait on the respective qT quarter's arrival.
            probe = constp.tile([1, 8], BF16, name="probe")
            nc.gpsimd.tensor_copy(probe, qT_s[0:1, 1, 0, 0:8])
            for kt in range(4):
                nc.gpsimd.dma_start(out=keep_s[:, kt, :], in_=keepT_d[:, kt, :])
            probe2 = constp.tile([1, 8], BF16, name="probe2")
            nc.gpsimd.tensor_copy(probe2, qT_s[0:1, 3, 0, 0:8])
            for kt in range(4, NKT):
                nc.gpsimd.dma_start(out=keep_s[:, kt, :], in_=keepT_d[:, kt, :])

            # ---- filler queue: PE work to interleave into attention ----
            filler = []
            fill_pos = [0]

            def pull(n):
                ran = 0
                while ran < n and fill_pos[0] < len(filler):
                    filler[fill_pos[0]][1]()
                    fill_pos[0] += 1
                    ran += 1

            def drain(stage):
                while fill_pos[0] < len(filler) and filler[fill_pos[0]][0] <= stage:
                    filler[fill_pos[0]][1]()
                    fill_pos[0] += 1

            evac_flip = [0]

            def evac(dst, src):
                if evac_flip[0] == 0:
                    nc.scalar.copy(dst, src)
                else:
                    nc.vector.tensor_copy(dst, src)
                evac_flip[0] ^= 1

            qkv = {}

            def enqueue_proj(h):
                QT_t = qkvp.tile([128, S], BF16, tag="QT", name=f"QT{h}")
                KT_t = qkvp.tile([128, S], BF16, tag="KT", name=f"KT{h}")
                V_t = qkvp.tile([128, NKT, 128], BF16, tag="V", name=f"V{h}")
                vt_t = vtp.tile([128, S], BF16, tag="vt", name=f"vt{h}")
                qkv[h] = (QT_t, KT_t, V_t)

                def chunk(wname, c, dst):
                    hold = {}
                    ths = []
                    for et in range(NET):
                        def th(et=et, wname=wname, c=c, dst=dst, hold=hold):
                            if et == 0:
                                hold["ps"] = ps_pr.tile(
                                    [128, 512], F32, tag="pr", name=f"pp{h}"
                                )
                            nc.tensor.matmul(
                                hold["ps"],
                                lhsT=w_s[wname][:, h, et, :],
                                rhs=qT_sl(c)[:, et, :],
                                start=(et == 0),
                                stop=(et == NET - 1),
                            )
                            if et == NET - 1:
                                evac(dst[:, c * 512 : (c + 1) * 512], hold["ps"])
                        ths.append(th)
                    return ths

                def pst_group(g):
                    hold = {}
                    ths = []
                    for j in range(8):
                        def th(j=j, g=g, hold=hold):
                            if j == 0:
                                hold["ps"] = ps_pr.tile(
                                    [128, 8, 128], BF16, tag="pr", name=f"pt{h}"
                                )
                            st = 8 * g + j
                            nc.tensor.transpose(
                                hold["ps"][:, j, :],
                                vt_t[:, st * 128 : (st + 1) * 128],
                                ident,
                            )
                            if j == 7:
                                nc.vector.tensor_copy(
                                    V_t[:, 8 * g : 8 * g + 8, :], hold["ps"]
                                )
                        ths.append(th)
                    return ths

                seq = []
                for c in range(4):
                    seq += chunk("wk", c, KT_t)
                    seq += chunk("wv", c, vt_t)
                    seq += chunk("wq", c, QT_t)
                    if c == 1:
                        seq += pst_group(0)
                seq += pst_group(1)
                for th in seq:
                    filler.append((h, th))

            def enqueue_out(qts, stage, wide):
                # wide=False: [128,512] chunks through the single-buffer proj
                # bank (safe to pull as filler inside attention pairs).
                # wide=True: [128,1024] through the double-buffered lg ring.
                # Evacuation is LAZY (emitted just before the next qt's
                # alloc) so the ring never blocks the next qt's matmuls.
                lazy = {}

                def flush():
                    if "po" in lazy:
                        po, qt_, half_, nmm_ = lazy.pop("po")
                        for m in range(nmm_):
                            e0 = half_ * (1024 if wide else 512) + m * 512
                            ob = obp.tile([128, 512], F32, tag="ob", name="ob")
                            evac(ob, po[:, m * 512 : (m + 1) * 512])
                            nc.sync.dma_start(
                                out=out_d[
                                    qt_ * 128 : (qt_ + 1) * 128, e0 : e0 + 512
                                ],
                                in_=ob,
                            )

                for qt in qts:
                    for half in range(1 if wide else 2):
                        hold = {}
                        for hh in range(HPC):
                            def th(hh=hh, qt=qt, half=half, hold=hold):
                                nmm = 2 if wide else 1
                                if hh == 0:
                                    flush()
                                    hold["po"] = (
                                        ps_lg.tile([128, 1024], F32, tag="lg", name="po")
                                        if wide
                                        else ps_pr.tile([128, 512], F32, tag="pr", name="po")
                                    )
                                for m in range(nmm):
                                    e0 = half * (1024 if wide else 512) + m * 512
                                    nc.tensor.matmul(
                                        hold["po"][:, m * 512 : (m + 1) * 512],
                                        lhsT=headsT_s[:, hh, qt * 128 : (qt + 1) * 128],
                                        rhs=wo_s[:, hh, e0 : e0 + 512],
                                        start=(hh == 0),
                                        stop=(hh == HPC - 1),
                                    )
                                if hh == HPC - 1:
                                    lazy["po"] = (hold["po"], qt, half, nmm)
                            filler.append((stage, th))
                filler.append((stage, flush))

            # ---- deferred softmax normalization (one pair behind) ----
            def emit_norm(pn):
                sm, avs0, avs1, hh, q0 = pn
                lns = smallp.tile([33, 512], F32, tag="lns", name="lns", bufs=1)
                nc.scalar.activation(lns, sm, LN)
                rcb = smallp.tile([33, 512], BF16, tag="rcb", name="rcb")
                nc.scalar.activation(rcb, lns, EXP, scale=-1.0)
                pb = ps_lg.tile([128, 2, 512], F32, tag="lg", name="pb")
                nc.tensor.matmul(
                    pb[:, 0, :], lhsT=ones33[0:1, :], rhs=rcb[0:1, :],
                    start=True, stop=True,
                )
                nc.tensor.matmul(
                    pb[:, 1, :], lhsT=ones33[32:33, :], rhs=rcb[32:33, :],
                    start=True, stop=True,
                )
                rb = smallp.tile([128, 2, 512], BF16, tag="rb", name="rb", bufs=1)
                nc.scalar.copy(rb, pb)
                nc.vector.tensor_mul(headsT_s[:, hh, q0 : q0 + 512], avs0, rb[:, 0, :])
                nc.vector.tensor_mul(
                    headsT_s[:, hh, q0 + 512 : q0 + 1024], avs1, rb[:, 1, :]
                )

            # the softmax denominator reduction (ones-matmuls -> sm) and the
            # normalization are BOTH deferred: ones(p) runs at the start of
            # pair p+1 (its accumulators are a full pair old, so the PE never
            # waits on the acc chains), norm(p) at the start of pair p+2.
            ones_pending = [None]  # (acc_d, acc_p, avs0, avs1, h, q0)
            norm_pending = [None]  # (sm, avs0, avs1, h, q0)

            def emit_ones(op):
                acc_d, acc_p, avs0, avs1, hh, q0 = op
                sm = ps_sm.tile([33, 512], F32, tag="sm", name="sm")
                for row0, sl in ((0, slice(0, 512)), (32, slice(512, 1024))):
                    srcs = [a[:, i, sl] for a in (acc_d, acc_p) if a is not None for i in range(2)]
                    for si, src in enumerate(srcs):
                        nc.tensor.matmul(
                            sm[row0 : row0 + 1, :], lhsT=ones_col, rhs=src,
                            start=(si == 0), stop=(si == len(srcs) - 1),
                        )
                return (sm, avs0, avs1, hh, q0)

            def attention_pair(h, pair):
                QT_t, KT_t, V_t = qkv[h]
                q0 = pair * 1024
                norm_next = None
                if ones_pending[0] is not None:
                    norm_next = emit_ones(ones_pending[0])
                    ones_pending[0] = None
                norm_now = norm_pending[0]  # emitted at kt==2, see below
                norm_pending[0] = norm_next
                if h == HPC - 1 and pair == 1:
                    # first half of the output projection needs the last
                    # head's pair-0 norm before its thunks become pullable
                    if norm_now is not None:
                        emit_norm(norm_now)
                        norm_now = None
                    enqueue_out(range(8), stage=HPC, wide=False)
                ex2_of = {}   # kt -> (ex2 tile, sub-index)
                ex2s = {}     # j -> ex2 tile
                acc_d = None
                acc_p = None
                av = [None]
                av_started = [False]
                av_queue = []  # (kt, due_iter)

                def emit_av(kt, stop):
                    if av[0] is None:
                        av[0] = ps_av.tile([128, 2, 512], F32, tag="av", name="av")
                    t, i = ex2_of[kt]
                    for sub in range(2):
                        nc.tensor.matmul(
                            av[0][:, sub, :],
                            lhsT=V_t[:, kt, :],
                            rhs=t[:, i, sub * 512 : (sub + 1) * 512],
                            start=not av_started[0],
                            stop=stop,
                        )
                    av_started[0] = True

                for kt in range(NKT):
                    if kt == 2 and norm_now is not None:
                        # deferred here so the norm's ACT ops queue behind
                        # exp(0..1) instead of blocking them at pair start
                        emit_norm(norm_now)
                        norm_now = None
                    j = kt // 2
                    lg = ps_lg.tile([128, 1024], F32, tag="lg", name="lg")
                    for half in range(2):
                        nc.tensor.matmul(
                            lg[:, half * 512 : (half + 1) * 512],
                            lhsT=KT_t[:, kt * 128 : (kt + 1) * 128],
                            rhs=QT_t[:, q0 + half * 512 : q0 + (half + 1) * 512],
                            start=True,
                            stop=True,
                        )
                    if kt % 2 == 0:
                        ex2s[j] = expp.tile([128, 2, 1024], BF16, tag="ex", name="ex")
                    ex2 = ex2s[j]
                    nc.scalar.activation(ex2[:, kt % 2, :], lg, EXP, scale=SCALE)
                    ex2_of[kt] = (ex2, kt % 2)
                    if kt % 2 == 1:
                        # one wide mask-mul covers both key tiles of this ex2
                        nc.vector.tensor_mul(
                            ex2, ex2, keep_s[:, kt - 1 : kt + 1, q0 : q0 + 1024]
                        )
                        if j in POOL_JS:
                            if acc_p is None:
                                acc_p = accp.tile(
                                    [128, 2, 1024], BF16, tag="accp", name="accp", bufs=1
                                )
                                nc.gpsimd.tensor_add(acc_p, ex2s[j - 1], ex2)
                            else:
                                nc.gpsimd.tensor_add(acc_p, acc_p, ex2)
                        elif (POOL_JS and j == min(POOL_JS) - 1) or j == 0:
                            pass  # consumed by its chain's init later
                        elif j == 1:
                            acc_d = accp.tile(
                                [128, 2, 1024], BF16, tag="accd", name="accd", bufs=1
                            )
                            nc.vector.tensor_add(acc_d, ex2s[0], ex2)
                        else:
                            nc.vector.tensor_add(acc_d, acc_d, ex2)
                    pull(2 if kt < 10 else 4)
                    while av_queue and av_queue[0][1] <= kt:
                        emit_av(av_queue.pop(0)[0], stop=False)
                    av_queue.append((kt, kt + 1 if kt % 2 else kt + 2))
                while len(av_queue) > 1:
                    emit_av(av_queue.pop(0)[0], stop=False)
                emit_av(av_queue.pop(0)[0], stop=True)
                pull(20)
                avs0 = avsp.tile([128, 512], BF16, tag="avs", name="avs0")
                nc.scalar.copy(avs0, av[0][:, 0, :])
                avs1 = avsp.tile([128, 512], BF16, tag="avs", name="avs1")
                nc.vector.tensor_copy(avs1, av[0][:, 1, :])
                pack = (acc_d, acc_p, avs0, avs1, h, q0)
                if 2 * h + pair == 6:
                    # pair 6 is (h3,p0): its sm must exist before pair 7's
                    # start (which enqueues the first output-proj batch), so
                    # run its ones at its own tail. Consume the pending norm
                    # first so the single sm PSUM slot is free.
                    if norm_pending[0] is not None:
                        emit_norm(norm_pending[0])
                    norm_pending[0] = emit_ones(pack)
                else:
                    ones_pending[0] = pack
                pull(2)

            # ---- the pipeline ----
            enqueue_proj(0)
            drain(0)
            for h in range(HPC):
                if h + 1 < HPC:
                    enqueue_proj(h + 1)
                for pair in (0, 1):
                    attention_pair(h, pair)
                if h + 1 < HPC:
                    drain(h + 1)
            # final pair's denominator + norm, overlapped with output tile
            # qt8: its heads 0..2 don't depend on the norm, so they run on
            # the PE while the serial ln/exp/broadcast chain executes; only
            # the hh3 matmuls wait. (The norm's pb broadcast takes the other
            # lg slot, whose previous occupant is already fully read.)
            np7 = emit_ones(ones_pending[0])
            po8 = ps_lg.tile([128, 1024], F32, tag="lg", name="po8")

            def qt8_mms(hhs, start, stop):
                for hh in hhs:
                    for m in range(2):
                        nc.tensor.matmul(
                            po8[:, m * 512 : (m + 1) * 512],
                            lhsT=headsT_s[:, hh, 8 * 128 : 9 * 128],
                            rhs=wo_s[:, hh, m * 512 : (m + 1) * 512],
                            start=start and hh == hhs[0],
                            stop=stop and hh == hhs[-1],
                        )

            qt8_mms([0, 1, 2], start=True, stop=False)
            emit_norm(np7)
            qt8_mms([3], start=False, stop=True)
            for m in range(2):
                ob8 = obp.tile([128, 512], F32, tag="ob", name="ob8")
                evac(ob8, po8[:, m * 512 : (m + 1) * 512])
                nc.sync.dma_start(
                    out=out_d[8 * 128 : 9 * 128, m * 512 : (m + 1) * 512], in_=ob8
                )
            enqueue_out(range(9, 16), stage=HPC + 1, wide=True)
            drain(HPC + 1)

    _split_waits(nc)
    _nc_cache = nc
    return nc


def kernel(q, mask, w_query, w_key, w_value, w_out):
    nc = _build_nc()
    bf16 = ml_dtypes.bfloat16

    # partition-major swizzles so every DMA is 128 contiguous big descriptors
    qTp = np.empty((B, 128, 4, NET, 512), dtype=bf16)
    keepTp = np.empty((B, 128, NKT, S), dtype=bf16)
    for b in range(B):
        qT = np.ascontiguousarray(q[b].T.astype(bf16))  # [E, S]
        qTp[b] = qT.reshape(NET, 128, 4, 512).transpose(1, 2, 0, 3)
        keepT = (~mask[b]).T.astype(bf16)  # [S(k), S(q)]
        keepTp[b] = keepT.reshape(NKT, 128, S).transpose(1, 0, 2)
    wp = {}
    for name, w in (("wq", w_query), ("wk", w_key), ("wv", w_value)):
        wp[name] = np.ascontiguousarray(
            w.astype(bf16).reshape(H, NET, 128, D).transpose(2, 0, 1, 3)
        )
    wop = np.ascontiguousarray(w_out.astype(bf16).transpose(1, 0, 2))  # [128, H, E]

    in_maps = []
    for c in range(NCORES):
        b, g = c // 2, c % 2
        hs = slice(g * HPC, (g + 1) * HPC)
        in_maps.append(
            {
                "qT": np.ascontiguousarray(qTp[b]),
                "keepT": np.ascontiguousarray(keepTp[b]),
                "wq": np.ascontiguousarray(wp["wq"][:, hs]),
                "wk": np.ascontiguousarray(wp["wk"][:, hs]),
                "wv": np.ascontiguousarray(wp["wv"][:, hs]),
                "wo": np.ascontiguousarray(wop[:, hs]),
            }
        )

    global _last_in_maps
    _last_in_maps = in_maps
    res = run_bass_kernel_spmd(nc, in_maps, list(range(NCORES)))
    outs = [r["out"] for r in res.results]
    return np.stack([outs[2 * b] + outs[2 * b + 1] for b in range(B)]).astype(
        np.float32
    )


# revision 65
# speedup vs baseline: 1.0329x; 1.0283x over previous
"""Multi-head self-attention on 8 trn2 NeuronCores.

Problem: B=4, S=2048, E=1024, H=8, D=128 MHA with a boolean attention mask.

Sharding: batch x head-group. Core c computes batch b=c//2 for heads
[4*(c%2), 4*(c%2)+4). Each core produces a partial output [S, E] (its 4
heads' contribution through w_out); the host sums the two partials per
batch. No on-device collectives needed.

Single interleaved pipeline (no phase barriers): projections of head h+1
and the output projection are software-pipelined into the attention of
head h as PE "filler" work, so the PE stays dense (which also keeps its
DVFS p-state at full clock). The softmax denominator is NOT computed
with ones-matmuls per key tile (a third of the attention PE time);
instead the masked exp tiles are accumulated across the 16 key tiles on
the DVE (bf16) and reduced with two tiny ones-matmuls per query-pair.

All DRAM tensors are pre-swizzled on the host into partition-major
layouts so every DMA is 128 contiguous descriptors of >=2KB (the
naive layouts were descriptor-bound at 256B/descriptor).

Per (head, query-pair of 1024), streaming over 16 key tiles of 128:
  lgT[128k, 1024q] = KT-tile.T @ QT        (PE)
  expT = exp(scale * lgT)                  (ACT, bf16 out)
  expT *= keepT-tile                       (DVE; masked keys -> 0)
  acc += expT                              (DVE only, bf16, wide 2-kt ops)
  av  += V-tile.T @ expT                   (PE, [128d, 512q] x2, one kt behind)
  tail: avs = av (ACT/DVE), sm = ones.T @ acc (PE, [1,512]x2 in one bank
    via partition offset 32); deferred one pair: rcb = exp(-ln(sm)) (ACT),
    pb = ones.T @ rcb broadcast (PE), headsT = avs * pb (DVE).
Output projection: out[128q, E] = sum_h headsT[h].T @ w_out[h], staged
through SBUF, DMA'd per [128, 512] chunk.

exp is computed without a running row-max: logits here are ~N(0, 2.7^2), so
exp stays well inside fp32 range and softmax is shift invariant.
"""

import math

import ml_dtypes
import numpy as np

import concourse.bass as bass
import concourse.tile as tile
from concourse import mybir
from concourse.bass_utils import run_bass_kernel_spmd
from concourse.masks import make_identity
from concourse.vector_clock import ScopedClock, VectorClock

B, S, E, H, D = 4, 2048, 1024, 8, 128
HPC = 4          # heads per core
NCORES = 8
NKT = S // 128   # key tiles per sequence
NET = E // 128   # contraction tiles for the projections
SCALE = 1.0 / math.sqrt(D)
BF16 = mybir.dt.bfloat16
F32 = mybir.dt.float32
EXP = mybir.ActivationFunctionType.Exp
LN = mybir.ActivationFunctionType.Ln

FILL_PER_KT = 3          # PE filler thunks pulled per key tile
# exp lands in double-wide [128, 2, 1024] tiles (one per 2 key tiles), so
# mask and acc run as half as many, twice as wide DVE ops. POOL_JS selects
# ex2 tiles whose acc-add would run on GpSimd — left EMPTY on purpose: a
# GpSimd wide tensor op running concurrently with a DVE mask slows BOTH ~3x
# via SBUF port contention (measured 1.2us -> 4.1us), which cascades into a
# PE stall; a single all-DVE chain is ~35us faster end-to-end.
POOL_JS = frozenset()

_patched = False


def _patch_drain():
    """The installed walrus rejects >1 sem wait on the Tile tail Drain.
    Emit one drain per pending logical processor instead."""
    global _patched
    if _patched:
        return
    _patched = True

    def _drain_and_barrier(self, tick_clock, wait_clock):
        nc = self.nc
        ticks = list(tick_clock.global_clock)
        procs = [i for i, t in enumerate(ticks) if t > 0]
        for p in procs or [None]:
            vec = [0] * len(ticks)
            if p is not None:
                vec[p] = ticks[p]
            d = nc.sync.drain()
            wait_clock.add_sem_waits(d.ins, ScopedClock({None: VectorClock(vec)}))
        nc.all_engine_barrier()
        popped = nc._tile_sem_poison_stack.pop()
        assert popped is self._sem_poison
        nc.clear_and_free_semaphores(list(self.sems.allocated().values()))
        nc.all_engine_barrier()

    tile.TileContext._drain_and_barrier = _drain_and_barrier


def _split_waits(nc):
    """This walrus build only encodes ONE sem wait per instruction. Move
    extra waits onto preceding same-engine NoOps (engines execute their
    instructions in block order, so this is semantically identical)."""
    import bass_rust

    k = 0
    for f in nc.m.functions:
        for bb in f.blocks:
            out = []
            for inst in bb.instructions:
                si = inst.sync_info
                if si is not None and si.on_wait and len(si.on_wait) > 1:
                    waits = list(si.on_wait)
                    for w in waits[:-1]:
                        nop = bass_rust.InstNoOp(
                            name=f"I-waitsplit-{k}", ins=[], outs=[]
                        )
                        k += 1
                        nop.engine = inst.engine
                        nop.sync_info = mybir.SyncInfo(on_wait=[w], on_update=[])
                        out.append(nop)
                    inst.sync_info = mybir.SyncInfo(
                        on_wait=[waits[-1]], on_update=si.on_update
                    )
                out.append(inst)
            bb.instructions[:] = out


_nc_cache = None


def _build_nc():
    global _nc_cache
    if _nc_cache is not None:
        return _nc_cache
    _patch_drain()

    nc = bass.Bass()
    # host-pre-swizzled, partition-major layouts (see kernel() below)
    qT_d = nc.declare_dram_parameter("qT", [128, 4, NET, 512], BF16, isOutput=False)
    keepT_d = nc.declare_dram_parameter("keepT", [128, NKT, S], BF16, isOutput=False)
    wq_d = nc.declare_dram_parameter("wq", [128, HPC, NET, D], BF16, isOutput=False)
    wk_d = nc.declare_dram_parameter("wk", [128, HPC, NET, D], BF16, isOutput=False)
    wv_d = nc.declare_dram_parameter("wv", [128, HPC, NET, D], BF16, isOutput=False)
    wo_d = nc.declare_dram_parameter("wo", [128, HPC, E], BF16, isOutput=False)
    out_d = nc.declare_dram_parameter("out", [S, E], F32, isOutput=True)
    w_d = {"wq": wq_d, "wk": wk_d, "wv": wv_d}

    with tile.TileContext(nc) as tc:
        with (
            tc.tile_pool(name="const", bufs=1) as constp,
            tc.tile_pool(name="wos", bufs=1) as wop,
            tc.tile_pool(name="hT", bufs=1) as hTp,
            tc.tile_pool(name="qTs", bufs=1) as qTp,
            tc.tile_pool(name="keep", bufs=1) as keepp,
            tc.tile_pool(name="ws", bufs=1) as wsp,
            tc.tile_pool(name="qkv", bufs=2) as qkvp,
            tc.tile_pool(name="vt", bufs=1) as vtp,
            tc.tile_pool(name="expt", bufs=3) as expp,
            tc.tile_pool(name="accs", bufs=2) as accp,
            tc.tile_pool(name="avs", bufs=4) as avsp,
            tc.tile_pool(name="small", bufs=2) as smallp,
            tc.tile_pool(name="obs", bufs=3) as obp,
            tc.tile_pool(name="ps_lg", bufs=2, space="PSUM") as ps_lg,
            tc.tile_pool(name="ps_av", bufs=1, space="PSUM") as ps_av,
            tc.tile_pool(name="ps_pr", bufs=1, space="PSUM") as ps_pr,
            tc.tile_pool(name="ps_sm", bufs=1, space="PSUM") as ps_sm,
        ):
            # ---- constants ----
            ident = constp.tile([128, 128], BF16)
            make_identity(nc, ident)
            ones_col = constp.tile([128, 1], BF16)
            nc.vector.memset(ones_col, 1.0)
            ones33 = constp.tile([33, 128], BF16)
            nc.vector.memset(ones33, 1.0)

            wo_s = wop.tile([128, HPC, E], BF16)
            headsT_s = hTp.tile([128, HPC, S], BF16)
            qT_s = qTp.tile([128, 4, NET, 512], BF16)
            keep_s = keepp.tile([128, NKT, S], BF16)
            w_s = {
                name: wsp.tile([128, HPC, NET, D], BF16, tag=name, name=name)
                for name in ("wq", "wk", "wv")
            }

            def qT_sl(c):
                # global s-columns [512c, 512c+512) in the quarter-major layout
                return qT_s[:, c]

            # ---- prefetch DMAs (sync queue: weights+qT+wo; pool queue: keepT)
            def load_w(name, h):
                nc.sync.dma_start(out=w_s[name][:, h], in_=w_d[name][:, h])

            load_w("wk", 0)
            nc.sync.dma_start(out=qT_s[:, 0], in_=qT_d[:, 0])
            load_w("wv", 0)
            load_w("wq", 0)
            for qu in range(1, 4):
                nc.sync.dma_start(out=qT_s[:, qu], in_=qT_d[:, qu])
            for h in range(1, HPC):
                for name in ("wk", "wv", "wq"):
                    load_w(name, h)
                if h == 1:
                    nc.sync.dma_start(out=wo_s, in_=wo_d[:, :, :])
            # hold the whole mask stream until the projection-critical qT
            # quarters are in (shared HBM bandwidth; keepT isn't needed until
            # attention starts ~45us in). The probes just make the Pool DMA
            # queue wait on the respective qT quarter's arrival.
            probe = constp.tile([1, 8], BF16, name="probe")
            nc.gpsimd.tensor_copy(probe, qT_s[0:1, 1, 0, 0:8])
            for kt in range(4):
                nc.gpsimd.dma_start(out=keep_s[:, kt, :], in_=keepT_d[:, kt, :])
            probe2 = constp.tile([1, 8], BF16, name="probe2")
            nc.gpsimd.tensor_copy(probe2, qT_s[0:1, 3, 0, 0:8])
            for kt in range(4, NKT):
                nc.gpsimd.dma_start(out=keep_s[:, kt, :], in_=keepT_d[:, kt, :])

            # ---- filler queue: PE work to interleave into attention ----
            filler = []
            fill_pos = [0]

            def pull(n):
                ran = 0
                while ran < n and fill_pos[0] < len(filler):
                    filler[fill_pos[0]][1]()
                    fill_pos[0] += 1
                    ran += 1

            def drain(stage):
                while fill_pos[0] < len(filler) and filler[fill_pos[0]][0] <= stage:
                    filler[fill_pos[0]][1]()
                    fill_pos[0] += 1

            evac_flip = [0]

            def evac(dst, src):
                if evac_flip[0] == 0:
                    nc.scalar.copy(dst, src)
                else:
                    nc.vector.tensor_copy(dst, src)
                evac_flip[0] ^= 1

            qkv = {}

            def enqueue_proj(h):
                QT_t = qkvp.tile([128, S], BF16, tag="QT", name=f"QT{h}")
                KT_t = qkvp.tile([128, S], BF16, tag="KT", name=f"KT{h}")
                V_t = qkvp.tile([128, NKT, 128], BF16, tag="V", name=f"V{h}")
                vt_t = vtp.tile([128, S], BF16, tag="vt", name=f"vt{h}")
                qkv[h] = (QT_t, KT_t, V_t)

                def chunk(wname, c, dst):
                    hold = {}
                    ths = []
                    for et in range(NET):
                        def th(et=et, wname=wname, c=c, dst=dst, hold=hold):
                            if et == 0:
                                hold["ps"] = ps_pr.tile(
                                    [128, 512], F32, tag="pr", name=f"pp{h}"
                                )
                            nc.tensor.matmul(
                                hold["ps"],
                                lhsT=w_s[wname][:, h, et, :],
                                rhs=qT_sl(c)[:, et, :],
                                start=(et == 0),
                                stop=(et == NET - 1),
                            )
                            if et == NET - 1:
                                evac(dst[:, c * 512 : (c + 1) * 512], hold["ps"])
                        ths.append(th)
                    return ths

                def pst_group(g):
                    hold = {}
                    ths = []
                    for j in range(8):
                        def th(j=j, g=g, hold=hold):
                            if j == 0:
                                hold["ps"] = ps_pr.tile(
                                    [128, 8, 128], BF16, tag="pr", name=f"pt{h}"
                                )
                            st = 8 * g + j
                            nc.tensor.transpose(
                                hold["ps"][:, j, :],
                                vt_t[:, st * 128 : (st + 1) * 128],
                                ident,
                            )
                            if j == 7:
                                nc.vector.tensor_copy(
                                    V_t[:, 8 * g : 8 * g + 8, :], hold["ps"]
                                )
                        ths.append(th)
                    return ths

                seq = []
                for c in range(4):
                    seq += chunk("wk", c, KT_t)
                    seq += chunk("wv", c, vt_t)
                    seq += chunk("wq", c, QT_t)
                    if c == 1:
                        seq += pst_group(0)
                seq += pst_group(1)
                for th in seq:
                    filler.append((h, th))

            def enqueue_out(qts, stage, wide):
                # wide=False: [128,512] chunks through the single-buffer proj
                # bank (safe to pull as filler inside attention pairs).
                # wide=True: [128,1024] through the double-buffered lg ring.
                # Evacuation is LAZY (emitted just before the next qt's
                # alloc) so the ring never blocks the next qt's matmuls.
                lazy = {}

                def flush():
                    if "po" in lazy:
                        po, qt_, half_, nmm_ = lazy.pop("po")
                        for m in range(nmm_):
                            e0 = half_ * (1024 if wide else 512) + m * 512
                            ob = obp.tile([128, 512], F32, tag="ob", name="ob")
                            evac(ob, po[:, m * 512 : (m + 1) * 512])
                            nc.sync.dma_start(
                                out=out_d[
                                    qt_ * 128 : (qt_ + 1) * 128, e0 : e0 + 512
                                ],
                                in_=ob,
                            )

                for qt in qts:
                    for half in range(1 if wide else 2):
                        hold = {}
                        for hh in range(HPC):
                            def th(hh=hh, qt=qt, half=half, hold=hold):
                                nmm = 2 if wide else 1
                                if hh == 0:
                                    flush()
                                    hold["po"] = (
                                        ps_lg.tile([128, 1024], F32, tag="lg", name="po")
                                        if wide
                                        else ps_pr.tile([128, 512], F32, tag="pr", name="po")
                                    )
                                for m in range(nmm):
                                    e0 = half * (1024 if wide else 512) + m * 512
                                    nc.tensor.matmul(
                                        hold["po"][:, m * 512 : (m + 1) * 512],
                                        lhsT=headsT_s[:, hh, qt * 128 : (qt + 1) * 128],
                                        rhs=wo_s[:, hh, e0 : e0 + 512],
                                        start=(hh == 0),
                                        stop=(hh == HPC - 1),
                                    )
                                if hh == HPC - 1:
                                    lazy["po"] = (hold["po"], qt, half, nmm)
                            filler.append((stage, th))
                filler.append((stage, flush))

            # ---- deferred softmax normalization (one pair behind) ----
            def emit_norm(pn):
                sm, avs0, avs1, hh, q0 = pn
                lns = smallp.tile([33, 512], F32, tag="lns", name="lns", bufs=1)
                nc.scalar.activation(lns, sm, LN)
                rcb = smallp.tile([33, 512], BF16, tag="rcb", name="rcb")
                nc.scalar.activation(rcb, lns, EXP, scale=-1.0)
                pb = ps_lg.tile([128, 2, 512], F32, tag="lg", name="pb")
                nc.tensor.matmul(
                    pb[:, 0, :], lhsT=ones33[0:1, :], rhs=rcb[0:1, :],
                    start=True, stop=True,
                )
                nc.tensor.matmul(
                    pb[:, 1, :], lhsT=ones33[32:33, :], rhs=rcb[32:33, :],
                    start=True, stop=True,
                )
                # muls read the f32 broadcast directly from PSUM: one less
                # serial stage (and one less rounding) in the norm chain
                nc.vector.tensor_mul(headsT_s[:, hh, q0 : q0 + 512], avs0, pb[:, 0, :])
                nc.vector.tensor_mul(
                    headsT_s[:, hh, q0 + 512 : q0 + 1024], avs1, pb[:, 1, :]
                )

            # the softmax denominator reduction (ones-matmuls -> sm) and the
            # normalization are BOTH deferred: ones(p) runs at the start of
            # pair p+1 (its accumulators are a full pair old, so the PE never
            # waits on the acc chains), norm(p) at the start of pair p+2.
            ones_pending = [None]  # (acc_d, acc_p, avs0, avs1, h, q0)
            norm_pending = [None]  # (sm, avs0, avs1, h, q0)

            def emit_ones(op):
                acc_d, acc_p, avs0, avs1, hh, q0 = op
                sm = ps_sm.tile([33, 512], F32, tag="sm", name="sm")
                for row0, sl in ((0, slice(0, 512)), (32, slice(512, 1024))):
                    srcs = [a[:, i, sl] for a in (acc_d, acc_p) if a is not None for i in range(2)]
                    for si, src in enumerate(srcs):
                        nc.tensor.matmul(
                            sm[row0 : row0 + 1, :], lhsT=ones_col, rhs=src,
                            start=(si == 0), stop=(si == len(srcs) - 1),
                        )
                return (sm, avs0, avs1, hh, q0)

            def attention_pair(h, pair):
                QT_t, KT_t, V_t = qkv[h]
                q0 = pair * 1024
                norm_next = None
                if ones_pending[0] is not None:
                    norm_next = emit_ones(ones_pending[0])
                    ones_pending[0] = None
                norm_now = norm_pending[0]  # emitted at kt==2, see below
                norm_pending[0] = norm_next
                if h == HPC - 1 and pair == 1:
                    # first half of the output projection needs the last
                    # head's pair-0 norm before its thunks become pullable
                    if norm_now is not None:
                        emit_norm(norm_now)
                        norm_now = None
                    enqueue_out(range(8), stage=HPC, wide=False)
                ex2_of = {}   # kt -> (ex2 tile, sub-index)
                ex2s = {}     # j -> ex2 tile
                acc_d = None
                acc_p = None
                av = [None]
                av_started = [False]
                av_queue = []  # (kt, due_iter)

                def emit_av(kt, stop):
                    if av[0] is None:
                        av[0] = ps_av.tile([128, 2, 512], F32, tag="av", name="av")
                    t, i = ex2_of[kt]
                    for sub in range(2):
                        nc.tensor.matmul(
                            av[0][:, sub, :],
                            lhsT=V_t[:, kt, :],
                            rhs=t[:, i, sub * 512 : (sub + 1) * 512],
                            start=not av_started[0],
                            stop=stop,
                        )
                    av_started[0] = True

                for kt in range(NKT):
                    if kt == 2 and norm_now is not None:
                        # deferred here so the norm's ACT ops queue behind
                        # exp(0..1) instead of blocking them at pair start
                        emit_norm(norm_now)
                        norm_now = None
                    j = kt // 2
                    lg = ps_lg.tile([128, 1024], F32, tag="lg", name="lg")
                    for half in range(2):
                        nc.tensor.matmul(
                            lg[:, half * 512 : (half + 1) * 512],
                            lhsT=KT_t[:, kt * 128 : (kt + 1) * 128],
                            rhs=QT_t[:, q0 + half * 512 : q0 + (half + 1) * 512],
                            start=True,
                            stop=True,
                        )
                    if kt % 2 == 0:
                        ex2s[j] = expp.tile([128, 2, 1024], BF16, tag="ex", name="ex")
                    ex2 = ex2s[j]
                    nc.scalar.activation(ex2[:, kt % 2, :], lg, EXP, scale=SCALE)
                    ex2_of[kt] = (ex2, kt % 2)
                    if kt % 2 == 1:
                        # one wide mask-mul covers both key tiles of this ex2
                        nc.vector.tensor_mul(
                            ex2, ex2, keep_s[:, kt - 1 : kt + 1, q0 : q0 + 1024]
                        )
                        if j in POOL_JS:
                            if acc_p is None:
                                acc_p = accp.tile(
                                    [128, 2, 1024], BF16, tag="accp", name="accp", bufs=1
                                )
                                nc.gpsimd.tensor_add(acc_p, ex2s[j - 1], ex2)
                            else:
                                nc.gpsimd.tensor_add(acc_p, acc_p, ex2)
                        elif (POOL_JS and j == min(POOL_JS) - 1) or j == 0:
                            pass  # consumed by its chain's init later
                        elif j == 1:
                            acc_d = accp.tile(
                                [128, 2, 1024], BF16, tag="accd", name="accd", bufs=1
                            )
                            nc.vector.tensor_add(acc_d, ex2s[0], ex2)
                        else:
                            nc.vector.tensor_add(acc_d, acc_d, ex2)
                    pull(2 if kt < 10 else 4)
                    while av_queue and av_queue[0][1] <= kt:
                        emit_av(av_queue.pop(0)[0], stop=False)
                    av_queue.append((kt, kt + 1 if kt % 2 else kt + 2))
                while len(av_queue) > 1:
                    emit_av(av_queue.pop(0)[0], stop=False)
                emit_av(av_queue.pop(0)[0], stop=True)
                pull(20)
                avs0 = avsp.tile([128, 512], BF16, tag="avs", name="avs0")
                nc.scalar.copy(avs0, av[0][:, 0, :])
                avs1 = avsp.tile([128, 512], BF16, tag="avs", name="avs1")
                nc.vector.tensor_copy(avs1, av[0][:, 1, :])
                pack = (acc_d, acc_p, avs0, avs1, h, q0)
                if 2 * h + pair == 6:
                    # pair 6 is (h3,p0): its sm must exist before pair 7's
                    # start (which enqueues the first output-proj batch), so
                    # run its ones at its own tail. Consume the pending norm
                    # first so the single sm PSUM slot is free.
                    if norm_pending[0] is not None:
                        emit_norm(norm_pending[0])
                    norm_pending[0] = emit_ones(pack)
                else:
                    ones_pending[0] = pack
                pull(2)

            # ---- the pipeline ----
            enqueue_proj(0)
            drain(0)
            for h in range(HPC):
                if h + 1 < HPC:
                    enqueue_proj(h + 1)
                for pair in (0, 1):
                    attention_pair(h, pair)
                if h + 1 < HPC:
                    drain(h + 1)
            # final pair's denominator + norm, overlapped with output tile
            # qt8: its heads 0..2 don't depend on the norm, so they run on
            # the PE while the serial ln/exp/broadcast chain executes; only
            # the hh3 matmuls wait. (The norm's pb broadcast takes the other
            # lg slot, whose previous occupant is already fully read.)
            np7 = emit_ones(ones_pending[0])
            po8 = ps_lg.tile([128, 1024], F32, tag="lg", name="po8")

            def qt8_mms(hhs, start, stop):
                for hh in hhs:
                    for m in range(2):
                        nc.tensor.matmul(
                            po8[:, m * 512 : (m + 1) * 512],
                            lhsT=headsT_s[:, hh, 8 * 128 : 9 * 128],
                            rhs=wo_s[:, hh, m * 512 : (m + 1) * 512],
                            start=start and hh == hhs[0],
                            stop=stop and hh == hhs[-1],
                        )

            qt8_mms([0, 1, 2], start=True, stop=False)
            emit_norm(np7)
            qt8_mms([3], start=False, stop=True)
            for m in range(2):
                ob8 = obp.tile([128, 512], F32, tag="ob", name="ob8")
                evac(ob8, po8[:, m * 512 : (m + 1) * 512])
                nc.sync.dma_start(
                    out=out_d[8 * 128 : 9 * 128, m * 512 : (m + 1) * 512], in_=ob8
                )
            enqueue_out(range(9, 16), stage=HPC + 1, wide=True)
            drain(HPC + 1)

    _split_waits(nc)
    _nc_cache = nc
    return nc


def kernel(q, mask, w_query, w_key, w_value, w_out):
    nc = _build_nc()
    bf16 = ml_dtypes.bfloat16

    # partition-major swizzles so every DMA is 128 contiguous big descriptors
    qTp = np.empty((B, 128, 4, NET, 512), dtype=bf16)
    keepTp = np.empty((B, 128, NKT, S), dtype=bf16)
    for b in range(B):
        qT = np.ascontiguousarray(q[b].T.astype(bf16))  # [E, S]
        qTp[b] = qT.reshape(NET, 128, 4, 512).transpose(1, 2, 0, 3)
        keepT = (~mask[b]).T.astype(bf16)  # [S(k), S(q)]
        keepTp[b] = keepT.reshape(NKT, 128, S).transpose(1, 0, 2)
    wp = {}
    for name, w in (("wq", w_query), ("wk", w_key), ("wv", w_value)):
        wp[name] = np.ascontiguousarray(
            w.astype(bf16).reshape(H, NET, 128, D).transpose(2, 0, 1, 3)
        )
    wop = np.ascontiguousarray(w_out.astype(bf16).transpose(1, 0, 2))  # [128, H, E]

    in_maps = []
    for c in range(NCORES):
        b, g = c // 2, c % 2
        hs = slice(g * HPC, (g + 1) * HPC)
        in_maps.append(
            {
                "qT": np.ascontiguousarray(qTp[b]),
                "keepT": np.ascontiguousarray(keepTp[b]),
                "wq": np.ascontiguousarray(wp["wq"][:, hs]),
                "wk": np.ascontiguousarray(wp["wk"][:, hs]),
                "wv": np.ascontiguousarray(wp["wv"][:, hs]),
                "wo": np.ascontiguousarray(wop[:, hs]),
            }
        )

    global _last_in_maps
    _last_in_maps = in_maps
    res = run_bass_kernel_spmd(nc, in_maps, list(range(NCORES)))
    outs = [r["out"] for r in res.results]
    return np.stack([outs[2 * b] + outs[2 * b + 1] for b in range(B)]).astype(
        np.float32
    )
